# revision 1
# baseline (speedup 1.0000x reference)
"""GAT-style attention (gnn_message_passing) Trainium2 kernel, 8-core row-parallel.

Math (algebraically identical to the reference masked-softmax attention):
  E = relu(h @ P)                [N,3]
  W = exp(E)
  denom[i,k] = sum_j A[i,j] W[j,k]
  out[i,:]   = rowsum[i] * sum_k (1/denom[i,k]) * sum_j A[i,j] W[j,k] h[j,:]
             = rowsum[i] * ((A * C) @ h)[i,:],  C[i,j] = sum_k W[j,k]/denom[i,k]

Two SPMD programs (collectives are unavailable on this runtime path, so the
tiny [4096,3] W matrix crosses cores via a host gather between programs):
  P1 (per core): W-shard = max(exp(relu(h_shard @ P) - 4ln2), 1/16)  [512,3]
  host: concatenate the 8 W-shards -> W_full [4096,3]  (pure data movement)
  P2 (per core): load A-shard cast to fp16, xbar-transpose to A.T;
      denom via PE (W'|1 stationary, i-sliced to pipeline with the A load);
      C.T tiles via PE (K=3); mask-multiply on DVE; main (A*C).T @ h on PE
      with h streaming during the loop; scale by rowsum/1024.
W is pre-scaled by 2^-4 and R by 1024 so everything fits fp16 range.
"""

import numpy as np

import concourse.bass as bass
import concourse.mybir as mybir
import concourse.tile as tile
from concourse import bacc
from concourse import bass_utils

N = 4096
D = 512
H = 3
NCORES = 8
SH = N // NCORES          # 512 output rows per core
JC = N // 128             # 32 j-chunks
IC = SH // 128            # 4 i-chunks
DC = D // 128             # 4 d-chunks
F16 = mybir.dt.float16
F32 = mybir.dt.float32
LN2x4 = float(4.0 * np.log(2.0))   # W scaled by 2^-4 to stay in fp16 range
RSCALE = 1024.0                    # R' = 1024/denom; undone in the final scale


def _body1(tc, h_shard_t, p_in, id_in, w_out):
    """P1: W-shard [SH,3] from h_shard.T [D,SH] and P."""
    nc = tc.nc
    with (
        tc.tile_pool(name="sb1", bufs=1) as sb,
        tc.tile_pool(name="ps1", bufs=2, space="PSUM") as ps,
    ):
        hst = sb.tile([128, DC * SH], F16, tag="hst")
        p16 = sb.tile([128, DC * H], F16, tag="p16")
        wsT = sb.tile([3, SH], F16, tag="wsT")
        bc = sb.tile([128, 12], F16, tag="bc")
        id16 = sb.tile([128, 128], F16, tag="id16")
        ebias = sb.tile([3, 1], F32, tag="ebias")
        nc.sync.dma_start(id16[:], id_in)
        nc.vector.memset(ebias[:], -LN2x4)

        nc.gpsimd.dma_start(
            out=hst[:].rearrange("p (dc j) -> p dc j", j=SH),
            in_=h_shard_t.rearrange("(dc p) j -> p dc j", p=128),
        )
        nc.gpsimd.dma_start(
            out=p16[:].rearrange("p (dc k) -> p dc k", k=H),
            in_=p_in.rearrange("(dc p) k -> p dc k", p=128),
        )
        psE = ps.tile([3, SH], F32, tag="scr", name="psE")
        for dc in range(DC):
            nc.tensor.matmul(
                psE[:],
                p16[:, dc * H:(dc + 1) * H],
                hst[:, dc * SH:(dc + 1) * SH],
                start=(dc == 0),
                stop=(dc == DC - 1),
            )
        nc.scalar.activation(
            wsT[:], psE[:], mybir.ActivationFunctionType.Exp,
            bias=ebias[:], scale=1.0,
        )
        nc.vector.tensor_scalar_max(wsT[:], wsT[:], 0.0625)
        psW = ps.tile([128, 16], F16, tag="scr", name="psW")
        for t in range(4):
            nc.tensor.transpose(
                psW[:, t * 4:t * 4 + 3],
                wsT[:, t * 128:(t + 1) * 128],
                id16[0:3, 0:3],
            )
        nc.vector.tensor_copy(
            bc[:].rearrange("p (t k) -> p t k", k=3),
            psW[:].rearrange("p (t s) -> p t s", s=4)[:, :, 0:3],
        )
        nc.sync.dma_start(
            out=w_out.rearrange("(t p) k -> p t k", p=128),
            in_=bc[:].rearrange("p (t k) -> p t k", k=3),
        )


def _body2(tc, a_rows, h_full, wt_in, w4_in, id_in, repl_in, out):
    """P2: the heavy pipeline. wt_in [3,N] / w4_in [128,JC*4] are host-layouts
    of the device-computed (scaled) W from P1."""
    nc = tc.nc
    mult = mybir.AluOpType.mult

    with (
        tc.tile_pool(name="big", bufs=1) as big,
        tc.tile_pool(name="small", bufs=1) as small,
        tc.tile_pool(name="mtp", bufs=4) as mtp,
        tc.tile_pool(name="osb", bufs=2) as osb,
        tc.tile_pool(name="psa", bufs=4, space="PSUM") as psa,
        tc.tile_pool(name="pso", bufs=1, space="PSUM") as pso,
    ):
        h16 = big.tile([128, JC * D], F16, tag="h16")       # h, j on partitions
        a16 = big.tile([128, IC * N], F16, tag="a16")       # A-shard natural
        at16 = big.tile([128, JC * SH], F16, tag="at16")    # A-shard transposed
        wT4 = small.tile([128, (JC // 4) * 128], F16, tag="wT4")  # W.T 4-row-packed
        repl = small.tile([3, 128], F16, tag="repl")        # R replication mm
        w4 = small.tile([128, JC * 4], F16, tag="w4")       # W'|ones (j on part)
        rT = small.tile([3, SH], F32, tag="rT")             # 1/denom (f32)
        rT16 = small.tile([3, SH], F16, tag="rT16")         # R' = 1024/denom
        rT16r = small.tile([128, SH], F16, tag="rT16r")     # R' at partitions 32t
        dn = small.tile([4, SH], F16, tag="dn")             # denom.T staging
        rs4 = small.tile([128, IC * 4], F32, tag="rs4")     # rowsum per-partition
        id16 = small.tile([128, 128], F16, tag="id16")

        nc.sync.dma_start(id16[:], id_in)

        # ---------------- loads ----------------
        # A first at full bandwidth (SWDGE cast, split for pipelining);
        # W tiles are fp16 already - plain HWDGE loads on a parallel queue;
        # h afterwards - it streams during the main loop. A is transposed on
        # the PE (identity matmuls) so the DMA engines never switch xbar mode
        # (transpose<->copy transitions serialize the whole DMA pipeline).
        a_r = a_rows.rearrange("(ic p) j -> ic p j", p=128)
        for ic in range(IC):
            for hh in range(2):
                nc.gpsimd.dma_start(
                    out=a16[:, ic * N + hh * (N // 2): ic * N + (hh + 1) * (N // 2)],
                    in_=a_r[ic][:, hh * (N // 2):(hh + 1) * (N // 2)],
                )
        nc.sync.dma_start(out=wT4[:], in_=wt_in)
        nc.sync.dma_start(out=w4[:], in_=w4_in)
        nc.sync.dma_start(out=repl[:], in_=repl_in)

        h_r = h_full.rearrange("(g jc p) d -> g p jc d", p=128, jc=4)
        h16_v = h16[:].rearrange("p (jc d) -> p jc d", d=D)
        for g in range(8):
            nc.gpsimd.dma_start(
                out=h16_v[:, g * 4:(g + 1) * 4, :], in_=h_r[g]
            )

        # ------ A transpose on PE (8 tiles per PSUM bank, ACT/DVE copies) ----
        at_v = at16[:].rearrange("p (jc i) -> p jc i", i=SH)
        for ic in range(IC):
            for g in range(JC // 8):
                xp = psa.tile([128, 8 * 128], F16, tag="scr", name=f"xp{ic}_{g}")
                for t in range(8):
                    jc = 8 * g + t
                    nc.tensor.transpose(
                        xp[:, t * 128:(t + 1) * 128],
                        a16[:, ic * N + jc * 128: ic * N + (jc + 1) * 128],
                        id16[:],
                    )
                dst = at_v[:, 8 * g:8 * (g + 1), ic * 128:(ic + 1) * 128]
                srcv = xp[:].rearrange("p (t i) -> p t i", i=128)
                if (ic * 4 + g) % 2 == 0:
                    nc.scalar.copy(dst, srcv)
                else:
                    nc.vector.tensor_copy(dst, srcv)

        # ------------- denominators (i-sliced to pipeline with transposes) ----
        psD = psa.tile([4, SH], F32, tag="scr", name="psD")
        for ic in range(IC):
            for jc in range(JC):
                nc.tensor.matmul(
                    psD[:, ic * 128:(ic + 1) * 128],
                    w4[:, jc * 4:(jc + 1) * 4],
                    at16[:, jc * SH + ic * 128: jc * SH + ic * 128 + 128],
                    start=(jc == 0),
                    stop=(jc == JC - 1),
                )

        nc.vector.reciprocal(rT[:], psD[0:3, :])
        nc.vector.tensor_scalar_mul(rT16[:], rT[:], RSCALE)
        psRep = psa.tile([128, SH], F32, tag="scr", name="psRep")
        nc.tensor.matmul(psRep[:], repl[:], rT16[:], start=True, stop=True)
        nc.vector.tensor_copy(rT16r[:], psRep[:])
        nc.vector.tensor_copy(dn[:], psD[:])
        psR = psa.tile([128, 16], F16, tag="scr", name="psR")
        for t in range(4):
            nc.tensor.transpose(
                psR[:, t * 4:(t + 1) * 4], dn[:, t * 128:(t + 1) * 128],
                id16[0:4, 0:4],
            )
        nc.scalar.activation(
            rs4[:], psR[:], mybir.ActivationFunctionType.Copy,
            bias=0.0, scale=1.0 / RSCALE,
        )

        # ---------------- main loop ----------------
        psO = [
            pso.tile([128, D], F32, tag=f"psO{ic}", name=f"psO{ic}")
            for ic in range(IC)
        ]
        for g in range(JC // 4):
            cts = []
            for t in range(4):
                ct = psa.tile([128, SH], F32, tag="scr", name=f"ct{g}_{t}")
                nc.tensor.matmul(
                    ct[:],
                    wT4[32 * t:32 * t + 3, g * 128:(g + 1) * 128],
                    rT16r[32 * t:32 * t + 3, :],
                    start=True,
                    stop=True,
                    tile_position=(32 * t, 0),
                )
                cts.append(ct)
            for t in range(4):
                jc = 4 * g + t
                mt = mtp.tile([128, SH], F16, tag="mt", name=f"mt{jc}")
                nc.vector.tensor_tensor(
                    mt[:], at16[:, jc * SH:(jc + 1) * SH], cts[t][:], op=mult
                )
                for ic in range(IC):
                    nc.tensor.matmul(
                        psO[ic][:],
                        mt[:, ic * 128:(ic + 1) * 128],
                        h16[:, jc * D:(jc + 1) * D],
                        start=(jc == 0),
                        stop=(jc == JC - 1),
                    )

        # ---------------- scale + store ----------------
        out_r = out.rearrange("(ic p) d -> ic p d", p=128)
        for ic in range(IC):
            ot = osb.tile([128, D], F32, tag="ot")
            nc.vector.tensor_scalar(
                ot[:], psO[ic][:], rs4[:, 4 * ic + 3: 4 * ic + 4], None, op0=mult
            )
            nc.sync.dma_start(out=out_r[ic], in_=ot[:])


_CACHE = {}


def _build1():
    if "p1" in _CACHE:
        return _CACHE["p1"]
    nc = bacc.Bacc("TRN2", target_bir_lowering=False, debug=False,
                   num_devices=NCORES)
    h_shard_t = nc.dram_tensor("h_shard_t", [D, SH], F32,
                               kind="ExternalInput").ap()
    p_in = nc.dram_tensor("p_in", [D, H], F32, kind="ExternalInput").ap()
    id_in = nc.dram_tensor("id_in", [128, 128], F16, kind="ExternalInput").ap()
    w_out = nc.dram_tensor("w_out", [SH, H], F16, kind="ExternalOutput").ap()
    with tile.TileContext(nc) as tc:
        _body1(tc, h_shard_t, p_in, id_in, w_out)
    nc.compile()
    _CACHE["p1"] = nc
    return nc


def _build2():
    if "p2" in _CACHE:
        return _CACHE["p2"]
    nc = bacc.Bacc("TRN2", target_bir_lowering=False, debug=False,
                   num_devices=NCORES)
    a_rows = nc.dram_tensor("a_rows", [SH, N], F32, kind="ExternalInput").ap()
    h_full = nc.dram_tensor("h_full", [N, D], F32, kind="ExternalInput").ap()
    wt_in = nc.dram_tensor("wt_in", [128, (JC // 4) * 128], F16,
                          kind="ExternalInput").ap()
    w4_in = nc.dram_tensor("w4_in", [128, JC * 4], F16,
                           kind="ExternalInput").ap()
    id_in = nc.dram_tensor("id_in", [128, 128], F16, kind="ExternalInput").ap()
    repl_in = nc.dram_tensor("repl_in", [3, 128], F16,
                             kind="ExternalInput").ap()
    out = nc.dram_tensor("out", [SH, D], F32, kind="ExternalOutput").ap()
    with tile.TileContext(nc) as tc:
        _body2(tc, a_rows, h_full, wt_in, w4_in, id_in, repl_in, out)
    nc.compile()
    _CACHE["p2"] = nc
    return nc


def kernel(graph_info, h, P, _trace=False, _results_out=None):
    graph_info = np.ascontiguousarray(graph_info, dtype=np.float32)
    h = np.ascontiguousarray(h, dtype=np.float32)
    P = np.ascontiguousarray(P, dtype=np.float32)
    nc1 = _build1()
    nc2 = _build2()

    id_host = np.eye(128, dtype=np.float16)
    in1 = [
        {
            "h_shard_t": np.ascontiguousarray(h[c * SH:(c + 1) * SH, :].T),
            "p_in": P,
            "id_in": id_host,
        }
        for c in range(NCORES)
    ]
    res1 = bass_utils.run_bass_kernel_spmd(
        nc1, in1, core_ids=list(range(NCORES)), trace=_trace
    )
    w_full = np.concatenate(
        [res1.results[c]["w_out"] for c in range(NCORES)], axis=0
    )
    # wt4: W.T packed so 4 consecutive j-chunks sit in row groups 32t..32t+2
    wr = w_full.reshape(JC // 4, 4, 128, H)      # [g, t, i, k]
    wt_host = np.zeros((128, (JC // 4) * 128), np.float16)
    for t in range(4):
        for k in range(H):
            wt_host[32 * t + k, :] = wr[:, t, :, k].reshape(-1)
    repl_host = np.zeros((3, 128), np.float16)
    for t in range(4):
        for k in range(H):
            repl_host[k, 32 * t + k] = 1.0
    w4_host = np.concatenate(
        [w_full.reshape(JC, 128, H).transpose(1, 0, 2),
         np.ones((128, JC, 1), np.float16)],
        axis=2,
    ).reshape(128, JC * 4)
    w4_host = np.ascontiguousarray(w4_host)

    in2 = [
        {
            "a_rows": graph_info[c * SH:(c + 1) * SH, :],
            "h_full": h,
            "wt_in": wt_host,
            "w4_in": w4_host,
            "id_in": id_host,
            "repl_in": repl_host,
        }
        for c in range(NCORES)
    ]
    res2 = bass_utils.run_bass_kernel_spmd(
        nc2, in2, core_ids=list(range(NCORES)), trace=_trace
    )
    if _results_out is not None:
        _results_out.extend([res1, res2])
    return np.concatenate(
        [res2.results[c]["out"] for c in range(NCORES)], axis=0
    )



# revision 25
# speedup vs baseline: 1.3557x; 1.3557x over previous
"""GAT-style attention (gnn_message_passing) Trainium2 kernel, 8-core row-parallel.

Math (identical to the reference masked-softmax attention):
  W' = max(exp(h @ P - 4ln2), 1/16)            [N,3]   (= exp(relu(h@P))/16)
  denom'[i,k] = sum_j A[i,j] W'[j,k]           (softmax denominators /16)
  rowsum[i]   = sum_j A[i,j]
  R[k,i]  = rowsum[i] / denom'[i,k] / 8        (rowsum folded in, /8 headroom)
  C[j,i]  = sum_k W'[j,k] R[k,i]               (PE, fp32 PSUM)
  mt[j,i] = (A.T[j,i] * 8) * C[j,i]            (mask-multiply, fp16)
  out[i,:] = sum_j mt[j,i] h[j,:]              (PE main loop)

Two SPMD programs (no collectives on this runtime path; the tiny [4096,3]
W' matrix crosses cores via a host gather between programs):
  P1: W'-shard from [h_shard.T | P] (single fp16 cast load, matmuls stream
      the 3-wide P side so PE time is negligible).
  P2: A arrives as a host COLUMN slice of graph_info (A.T layout on HBM),
      cast to fp8 on load (exact for a 0/1 mask) - no on-chip transposes.
      Denominators accumulate incrementally as A.T tiles land, streaming
      the 4-wide W'|1 side (N=4 matmuls). Main loop: C tiles on PE, masked
      multiply on DVE, aggregation matmuls stream h (fp16). Output stored
      fp16 and upcast on the host.
"""

import numpy as np

import concourse.bass as bass
import concourse.mybir as mybir
import concourse.tile as tile
from concourse import bacc
from concourse import bass_utils

N = 4096
D = 512
H = 3
NCORES = 8
SH = N // NCORES          # 512 output rows per core
JC = N // 128             # 32 j-chunks
IC = SH // 128            # 4 i-chunks
DC = D // 128             # 4 d-chunks
F8 = mybir.dt.float8e4
F16 = mybir.dt.float16
F32 = mybir.dt.float32
LN2x4 = float(4.0 * np.log(2.0))   # W scaled by 2^-4 to stay in fp16 range
HPW = 520                          # hp row width: 512 h cols + 3 P cols + pad
MTS = 8.0                          # mask scale (R carries 1/8)
N_JUNK = 70                        # PE p-state warmup transposes in P2

mult = mybir.AluOpType.mult


def _body1(tc, hp_in, w_out):
    """P1: W'-shard [SH,3] from hp = [h_shard.T | P] ([D, HPW] fp32)."""
    nc = tc.nc
    with (
        tc.tile_pool(name="sb1", bufs=1) as sb,
        tc.tile_pool(name="ps1", bufs=1, space="PSUM") as ps,
    ):
        hp = sb.tile([128, DC * HPW], F16, tag="hp")
        ws = sb.tile([128, 12], F16, tag="ws")
        ebias = sb.tile([128, 1], F32, tag="ebias")
        nc.vector.memset(ebias[:], -LN2x4)
        nc.gpsimd.dma_start(
            out=hp[:].rearrange("p (dc w) -> p dc w", w=HPW),
            in_=hp_in.rearrange("(dc p) w -> p dc w", p=128),
        )
        # jc-outer keeps each accumulation group's visits contiguous
        # (interleaved groups on one PSUM tile accumulate wrongly).
        psE = ps.tile([128, 12], F32, tag="psE", name="psE")
        for jc in range(4):
            for dc in range(DC):
                nc.tensor.matmul(
                    psE[:, 3 * jc:3 * jc + 3],
                    hp[:, dc * HPW + jc * 128: dc * HPW + (jc + 1) * 128],
                    hp[:, dc * HPW + 512: dc * HPW + 515],
                    start=(dc == 0),
                    stop=(dc == DC - 1),
                )
        nc.scalar.activation(
            ws[:], psE[:], mybir.ActivationFunctionType.Exp,
            bias=ebias[:], scale=1.0,
        )
        nc.vector.tensor_scalar_max(ws[:], ws[:], 0.0625)
        # w_out is [128, 12] packed (p, jc, k) - the host unpacks
        nc.sync.dma_start(out=w_out, in_=ws[:])


def _body2(tc, at_in, h_in, w4_in, wt_in, id_in, out):
    """P2: the heavy pipeline. at_in [N, SH] is the host column-slice of
    graph_info (A.T for this core's output rows)."""
    nc = tc.nc
    with (
        tc.tile_pool(name="big", bufs=1) as big,
        tc.tile_pool(name="small", bufs=1) as small,
        tc.tile_pool(name="mtp", bufs=6) as mtp,
        tc.tile_pool(name="osb", bufs=1) as osb,
        tc.tile_pool(name="ctp", bufs=2, space="PSUM") as ctp,
        tc.tile_pool(name="psd", bufs=1, space="PSUM") as psd,
        tc.tile_pool(name="pso", bufs=1, space="PSUM") as pso,
    ):
        at8 = big.tile([128, JC * SH], F8, tag="at8")     # A.T, j on partitions
        h16 = big.tile([128, JC * D], F16, tag="h16")     # h, j on partitions
        w4 = small.tile([128, JC * 4], F16, tag="w4")     # W'|1, j on partitions
        wt = small.tile([4, N], F16, tag="wt")            # W'.T
        id32 = small.tile([128, 128], F32, tag="id32")
        junk = small.tile([128, 128], F32, tag="junk")
        rc32 = small.tile([128, 16], F32, tag="rc32")     # 1/denom
        rn32 = small.tile([128, 16], F32, tag="rn32")     # rowsum/denom/8
        rT16 = small.tile([4, SH], F16, tag="rT16")       # R, k on partitions

        # PSUM is 8 banks, one tile per bank. Interleaved matmul accumulation
        # groups must live in separate PSUM tiles (column-sliced groups on one
        # tile accumulate wrongly), so: 2 denominator tiles used in two
        # passes, psR borrows a ct-pool rotation slot, 4 psO accumulators.
        psDn = [
            psd.tile([128, 4], F32, tag=f"psDn{i}", name=f"psDn{i}")
            for i in range(2)
        ]
        psR = ctp.tile([128, SH], F32, tag="ct", name="ctR")
        psO = [
            pso.tile([128, D], F32, tag=f"psO{ic}", name=f"psO{ic}")
            for ic in range(IC)
        ]

        # ---- tiny loads first (HWDGE; they run before the big SWDGE xfers)
        nc.sync.dma_start(out=w4[:], in_=w4_in)
        nc.sync.dma_start(out=wt[:], in_=wt_in)
        nc.sync.dma_start(out=id32[:], in_=id_in)
        nc.vector.memset(junk[:], 0.0)

        # ---- A.T cast load (fp32 -> fp8, exact for 0/1)
        at_v = at8[:].rearrange("p (jc i) -> p jc i", i=SH)
        at_groups = [(0, 6), (6, 14), (14, 23), (23, 32)]
        for lo, hi in at_groups:
            nc.gpsimd.dma_start(
                out=at_v[:, lo:hi, :],
                in_=at_in[lo * 128:hi * 128, :].rearrange(
                    "(jc p) i -> p jc i", p=128),
            )
        # ---- h cast load (fp32 -> fp16), first calls smaller for fast start
        h_v = h16[:].rearrange("p (jc d) -> p jc d", d=D)
        h_groups = [(0, 4), (4, 8), (8, 16), (16, 24), (24, 32)]
        for lo, hi in h_groups:
            nc.gpsimd.dma_start(
                out=h_v[:, lo:hi, :],
                in_=h_in[lo * 128:hi * 128, :].rearrange(
                    "(jc p) d -> p jc d", p=128),
            )

        def junk_mm(target=None):
            # p-state filler. Early bridges may scribble on psR (overwritten
            # by the real transposes later); late bridges use dead psO
            # corners (reset by the main loop's start=True matmuls).
            dst = psR if target is None else target
            nc.tensor.transpose(
                dst[0:4, 0:64], junk[:, 0:4], junk[:, 0:64],
            )

        def denom_wave(glo, ghi, ics):
            for jc in range(glo, ghi):
                for slot, ic in enumerate(ics):
                    nc.tensor.matmul(
                        psDn[slot][:],
                        at8[:, jc * SH + ic * 128: jc * SH + (ic + 1) * 128],
                        w4[:, 4 * jc:4 * jc + 4],
                        start=(jc == 0),
                        stop=(jc == JC - 1),
                    )

        def denom_finish_dve(ics):
            # R = rowsum/denom/8 for these i-chunks; frees the psDn tiles
            for slot, ic in enumerate(ics):
                nc.vector.reciprocal(rc32[:, 4 * ic:4 * ic + 4], psDn[slot][:])
                nc.vector.tensor_scalar(
                    rn32[:, 4 * ic:4 * ic + 4], rc32[:, 4 * ic:4 * ic + 4],
                    psDn[slot][:, 3:4], 1.0 / MTS,
                    op0=mult, op1=mult,
                )

        def denom_finish_pe(ics):
            for ic in ics:
                nc.tensor.transpose(
                    psR[0:4, ic * 128:(ic + 1) * 128],
                    rn32[:, 4 * ic:4 * ic + 4], id32[:],
                )

        # ---- PE p-state warmup junk bridges the dependency gaps so the PE
        # busy-streak is continuous from the last load wave through ct0
        # (3us of continuous PE busy => full 2.4GHz for the main loop).
        # Pass A accumulates i-chunks 0,1 incrementally as A.T tiles land;
        # pass B reuses the two PSUM tiles for i-chunks 2,3.
        for t in range(30):
            junk_mm()
        denom_wave(0, 6, (0, 1))
        for t in range(20):
            junk_mm()
        denom_wave(6, 14, (0, 1))
        for t in range(24):
            junk_mm()
        denom_wave(14, 23, (0, 1))
        for t in range(28):
            junk_mm()
        denom_wave(23, 32, (0, 1))
        denom_finish_dve((0, 1))
        for t in range(10):
            junk_mm()
        denom_wave(0, 32, (2, 3))
        denom_finish_pe((0, 1))
        denom_finish_dve((2, 3))
        for t in range(8):
            junk_mm(psO[0])
        denom_finish_pe((2, 3))
        for t in range(14):
            junk_mm(psO[1])
        nc.scalar.copy(rT16[:], psR[0:4, :])

        # ---- main loop: C tiles (PE) run 2 iterations ahead of the
        # mask-multiply (DVE) and aggregation matmuls so the PE never waits
        # on the DVE round-trip.
        LOOKAHEAD = 2
        cts = {}

        def emit_ct(jc):
            ct = ctp.tile([128, SH], F32, tag="ct", name=f"ct{jc}")
            nc.tensor.matmul(
                ct[:], wt[0:3, jc * 128:(jc + 1) * 128], rT16[0:3, :],
                start=True, stop=True,
            )
            cts[jc] = ct

        for jc in range(LOOKAHEAD):
            emit_ct(jc)
        for t in range(14):
            junk_mm(psO[2])
        for jc in range(JC):
            if jc + LOOKAHEAD < JC:
                emit_ct(jc + LOOKAHEAD)
            mt = mtp.tile([128, SH], F16, tag="mt", name=f"mt{jc}")
            nc.vector.scalar_tensor_tensor(
                mt[:], at8[:, jc * SH:(jc + 1) * SH], MTS, cts.pop(jc)[:],
                op0=mult, op1=mult,
            )
            for ic in range(IC):
                nc.tensor.matmul(
                    psO[ic][:],
                    mt[:, ic * 128:(ic + 1) * 128],
                    h16[:, jc * D:(jc + 1) * D],
                    start=(jc == 0),
                    stop=(jc == JC - 1),
                )

        # ---- store (fp16; host upcasts). Copies split ACT/DVE; the single
        # batched DMA lives on SP whose SEQ has no later work to block.
        ot = osb.tile([128, IC * D], F16, tag="ot")
        for ic in range(IC):
            dst = ot[:, ic * D:(ic + 1) * D]
            if ic % 2 == 0:
                nc.scalar.copy(dst, psO[ic][:])
            else:
                nc.vector.tensor_copy(dst, psO[ic][:])
        nc.sync.dma_start(
            out=out.rearrange("(ic p) d -> p ic d", p=128),
            in_=ot[:].rearrange("p (ic d) -> p ic d", d=D),
        )


_CACHE = {}


def _build1():
    if "p1" in _CACHE:
        return _CACHE["p1"]
    nc = bacc.Bacc("TRN2", target_bir_lowering=False, debug=False,
                   num_devices=NCORES)
    hp_in = nc.dram_tensor("hp_in", [D, HPW], F32, kind="ExternalInput").ap()
    w_out = nc.dram_tensor("w_out", [128, 12], F16, kind="ExternalOutput").ap()
    with tile.TileContext(nc) as tc:
        _body1(tc, hp_in, w_out)
    nc.compile()
    _CACHE["p1"] = nc
    return nc


def _build2():
    if "p2" in _CACHE:
        return _CACHE["p2"]
    nc = bacc.Bacc("TRN2", target_bir_lowering=False, debug=False,
                   num_devices=NCORES)
    at_in = nc.dram_tensor("at_in", [N, SH], F32, kind="ExternalInput").ap()
    h_in = nc.dram_tensor("h_in", [N, D], F32, kind="ExternalInput").ap()
    w4_in = nc.dram_tensor("w4_in", [128, JC * 4], F16,
                           kind="ExternalInput").ap()
    wt_in = nc.dram_tensor("wt_in", [4, N], F16, kind="ExternalInput").ap()
    id_in = nc.dram_tensor("id_in", [128, 128], F32, kind="ExternalInput").ap()
    out = nc.dram_tensor("out", [SH, D], F16, kind="ExternalOutput").ap()
    with tile.TileContext(nc) as tc:
        _body2(tc, at_in, h_in, w4_in, wt_in, id_in, out)
    nc.compile()
    _CACHE["p2"] = nc
    return nc


def kernel(graph_info, h, P, _trace=False, _results_out=None):
    graph_info = np.ascontiguousarray(graph_info, dtype=np.float32)
    h = np.ascontiguousarray(h, dtype=np.float32)
    P = np.ascontiguousarray(P, dtype=np.float32)
    nc1 = _build1()
    nc2 = _build2()

    # P1: hp = [h_shard.T | P | pad]
    pad = np.zeros((D, HPW - 512 - H), np.float32)
    in1 = [
        {"hp_in": np.ascontiguousarray(
            np.concatenate([h[c * SH:(c + 1) * SH, :].T, P, pad], axis=1))}
        for c in range(NCORES)
    ]
    res1 = bass_utils.run_bass_kernel_spmd(
        nc1, in1, core_ids=list(range(NCORES)), trace=_trace
    )
    w_full = np.concatenate(
        [res1.results[c]["w_out"].reshape(128, 4, 3).transpose(1, 0, 2)
         .reshape(SH, H) for c in range(NCORES)],
        axis=0,
    )

    # host packing of the tiny W' tables (pure data movement)
    w4_host = np.concatenate(
        [w_full.reshape(JC, 128, H).transpose(1, 0, 2),
         np.ones((128, JC, 1), np.float16)],
        axis=2,
    ).reshape(128, JC * 4)
    w4_host = np.ascontiguousarray(w4_host)
    wt_host = np.zeros((4, N), np.float16)
    wt_host[0:3, :] = w_full.T
    id_host = np.eye(128, dtype=np.float32)

    in2 = [
        {
            "at_in": np.ascontiguousarray(graph_info[c * SH:(c + 1) * SH, :].T),
            "h_in": h,
            "w4_in": w4_host,
            "wt_in": wt_host,
            "id_in": id_host,
        }
        for c in range(NCORES)
    ]
    res2 = bass_utils.run_bass_kernel_spmd(
        nc2, in2, core_ids=list(range(NCORES)), trace=_trace
    )
    if _results_out is not None:
        _results_out.extend([res1, res2])
    return np.concatenate(
        [res2.results[c]["out"].astype(np.float32) for c in range(NCORES)],
        axis=0,
    )


# revision 32
# speedup vs baseline: 1.3695x; 1.0102x over previous
"""GAT-style attention (gnn_message_passing) Trainium2 kernel, 8-core row-parallel.

Math (identical to the reference masked-softmax attention):
  W' = max(exp(h @ P - 4ln2), 1/16)            [N,3]   (= exp(relu(h@P))/16)
  denom'[i,k] = sum_j A[i,j] W'[j,k]           (softmax denominators /16)
  rowsum[i]   = sum_j A[i,j]
  R[k,i]  = rowsum[i] / denom'[i,k] / 8        (rowsum folded in, /8 headroom)
  C[j,i]  = sum_k W'[j,k] R[k,i]               (PE, fp32 PSUM)
  mt[j,i] = (A.T[j,i] * 8) * C[j,i]            (mask-multiply, fp16)
  out[i,:] = sum_j mt[j,i] h[j,:]              (PE main loop)

Two SPMD programs (no collectives on this runtime path; the tiny [4096,3]
W' matrix crosses cores via a host gather between programs):
  P1: W'-shard from [h_shard.T | P] (single fp16 cast load, matmuls stream
      the 3-wide P side so PE time is negligible).
  P2: A arrives as a host COLUMN slice of graph_info (A.T layout on HBM),
      cast to fp8 on load (exact for a 0/1 mask) - no on-chip transposes.
      Denominators accumulate incrementally as A.T tiles land, streaming
      the 4-wide W'|1 side (N=4 matmuls). Main loop: C tiles on PE, masked
      multiply on DVE, aggregation matmuls stream h (fp16). Output stored
      fp16 and upcast on the host.
"""

import numpy as np

import concourse.bass as bass
import concourse.mybir as mybir
import concourse.tile as tile
from concourse import bacc
from concourse import bass_utils

N = 4096
D = 512
H = 3
NCORES = 8
SH = N // NCORES          # 512 output rows per core
JC = N // 128             # 32 j-chunks
IC = SH // 128            # 4 i-chunks
DC = D // 128             # 4 d-chunks
F8 = mybir.dt.float8e4
F16 = mybir.dt.float16
F32 = mybir.dt.float32
LN2x4 = float(4.0 * np.log(2.0))   # W scaled by 2^-4 to stay in fp16 range
HPW = 520                          # hp row width: 512 h cols + 3 P cols + pad
MTS = 8.0                          # mask scale (R carries 1/8)
N_JUNK = 70                        # PE p-state warmup transposes in P2

mult = mybir.AluOpType.mult


def _body1(tc, hp_in, w_out):
    """P1: W'-shard [SH,3] from hp = [h_shard.T | P] ([D, HPW] fp32)."""
    nc = tc.nc
    with (
        tc.tile_pool(name="sb1", bufs=1) as sb,
        tc.tile_pool(name="ps1", bufs=1, space="PSUM") as ps,
    ):
        hp = sb.tile([128, DC * HPW], F16, tag="hp")
        ws = sb.tile([128, 12], F16, tag="ws")
        ebias = sb.tile([128, 1], F32, tag="ebias")
        nc.vector.memset(ebias[:], -LN2x4)
        nc.gpsimd.dma_start(
            out=hp[:].rearrange("p (dc w) -> p dc w", w=HPW),
            in_=hp_in.rearrange("(dc p) w -> p dc w", p=128),
        )
        # jc-outer keeps each accumulation group's visits contiguous
        # (interleaved groups on one PSUM tile accumulate wrongly).
        psE = ps.tile([128, 12], F32, tag="psE", name="psE")
        for jc in range(4):
            for dc in range(DC):
                nc.tensor.matmul(
                    psE[:, 3 * jc:3 * jc + 3],
                    hp[:, dc * HPW + jc * 128: dc * HPW + (jc + 1) * 128],
                    hp[:, dc * HPW + 512: dc * HPW + 515],
                    start=(dc == 0),
                    stop=(dc == DC - 1),
                )
        nc.scalar.activation(
            ws[:], psE[:], mybir.ActivationFunctionType.Exp,
            bias=ebias[:], scale=1.0,
        )
        # (the max(., 1/16) relu-equivalent is folded into P2's table prep)
        # w_out is [128, 12] packed (p, jc, k) - the host unpacks
        nc.sync.dma_start(out=w_out, in_=ws[:])


def _body2(tc, at_in, h_in, w4_in, wt_in, id_in, out):
    """P2: the heavy pipeline. at_in [N, SH] is the host column-slice of
    graph_info (A.T for this core's output rows)."""
    nc = tc.nc
    with (
        tc.tile_pool(name="big", bufs=1) as big,
        tc.tile_pool(name="small", bufs=1) as small,
        tc.tile_pool(name="mtp", bufs=6) as mtp,
        tc.tile_pool(name="osb", bufs=1) as osb,
        tc.tile_pool(name="ctp", bufs=2, space="PSUM") as ctp,
        tc.tile_pool(name="psd", bufs=1, space="PSUM") as psd,
        tc.tile_pool(name="pso", bufs=1, space="PSUM") as pso,
    ):
        at8 = big.tile([128, JC * SH], F8, tag="at8")     # A.T, j on partitions
        h16 = big.tile([128, JC * D], F16, tag="h16")     # h, j on partitions
        w4 = small.tile([128, JC * 4], F16, tag="w4")     # W'|1, j on partitions
        wt = small.tile([4, N], F16, tag="wt")            # W'.T
        id32 = small.tile([128, 128], F32, tag="id32")
        junk = small.tile([128, 128], F32, tag="junk")
        rc32 = small.tile([128, 16], F32, tag="rc32")     # 1/denom
        rn32 = small.tile([128, 16], F32, tag="rn32")     # rowsum/denom/8
        rT16 = small.tile([4, SH], F16, tag="rT16")       # R, k on partitions

        # PSUM is 8 banks, one tile per bank. Interleaved matmul accumulation
        # groups must live in separate PSUM tiles (column-sliced groups on one
        # tile accumulate wrongly), so: 2 denominator tiles used in two
        # passes, psR borrows a ct-pool rotation slot, 4 psO accumulators.
        psDn = [
            psd.tile([128, 4], F32, tag=f"psDn{i}", name=f"psDn{i}")
            for i in range(2)
        ]
        psR = ctp.tile([128, SH], F32, tag="ct", name="ctR")
        psO = [
            pso.tile([128, D], F32, tag=f"psO{ic}", name=f"psO{ic}")
            for ic in range(IC)
        ]

        # ---- tiny loads first (HWDGE; they run before the big SWDGE xfers)
        nc.sync.dma_start(out=w4[:], in_=w4_in)
        nc.sync.dma_start(out=wt[:], in_=wt_in)
        nc.sync.dma_start(out=id32[:], in_=id_in)
        nc.vector.memset(junk[:], 0.0)

        # ---- A.T cast load (fp32 -> fp8, exact for 0/1)
        at_v = at8[:].rearrange("p (jc i) -> p jc i", i=SH)
        at_groups = [(0, 6), (6, 14), (14, 23), (23, 32)]
        for lo, hi in at_groups:
            nc.gpsimd.dma_start(
                out=at_v[:, lo:hi, :],
                in_=at_in[lo * 128:hi * 128, :].rearrange(
                    "(jc p) i -> p jc i", p=128),
            )
        # ---- h cast load (fp32 -> fp16), first calls smaller for fast start
        h_v = h16[:].rearrange("p (jc d) -> p jc d", d=D)
        h_groups = [(0, 4), (4, 8), (8, 16), (16, 24), (24, 32)]
        for lo, hi in h_groups:
            nc.gpsimd.dma_start(
                out=h_v[:, lo:hi, :],
                in_=h_in[lo * 128:hi * 128, :].rearrange(
                    "(jc p) d -> p jc d", p=128),
            )

        def junk_mm(target=None):
            # p-state filler. Early bridges may scribble on psR (overwritten
            # by the real transposes later); late bridges use dead psO
            # corners (reset by the main loop's start=True matmuls).
            dst = psR if target is None else target
            nc.tensor.transpose(
                dst[0:4, 0:64], junk[:, 0:4], junk[:, 0:64],
            )

        # max(., 1/16) (the relu of exp(relu(.))) is applied here instead of
        # in P1 - off the critical path, right after the W tables land.
        nc.vector.tensor_scalar_max(w4[:], w4[:], 0.0625)
        nc.vector.tensor_scalar_max(wt[0:3, :], wt[0:3, :], 0.0625)

        # Denominator accumulators: 2 dedicated PSUM tiles + corners of
        # psO[2]/psO[3] (dead until the main loop's start=True resets them).
        # All 4 i-chunk groups accumulate in ONE pass as A.T tiles land.
        dslot = [psDn[0][:], psDn[1][:], psO[2][:, 0:4], psO[3][:, 0:4]]

        def denom_wave(glo, ghi):
            for jc in range(glo, ghi):
                for ic in range(IC):
                    nc.tensor.matmul(
                        dslot[ic],
                        at8[:, jc * SH + ic * 128: jc * SH + (ic + 1) * 128],
                        w4[:, 4 * jc:4 * jc + 4],
                        start=(jc == 0),
                        stop=(jc == JC - 1),
                    )

        # ---- PE p-state warmup junk bridges the dependency gaps so the PE
        # busy-streak is continuous from the last load wave through ct0
        # (3us of continuous PE busy => full 2.4GHz for the main loop).
        for t in range(30):
            junk_mm()
        denom_wave(0, 6)
        for t in range(20):
            junk_mm()
        denom_wave(6, 14)
        for t in range(24):
            junk_mm()
        denom_wave(14, 23)
        for t in range(28):
            junk_mm()
        denom_wave(23, 32)
        # R = rowsum/denom/8 for all i-chunks
        for ic in range(IC):
            nc.vector.reciprocal(rc32[:, 4 * ic:4 * ic + 4], dslot[ic])
            nc.vector.tensor_scalar(
                rn32[:, 4 * ic:4 * ic + 4], rc32[:, 4 * ic:4 * ic + 4],
                dslot[ic][:, 3:4], 1.0 / MTS,
                op0=mult, op1=mult,
            )
        for t in range(8):
            junk_mm()
        # transposed R goes to 4 separate dead psO corners so the per-chunk
        # ACT copies pipeline instead of serializing on one PSUM tile
        for ic in range(IC):
            nc.tensor.transpose(
                psO[ic][0:4, 0:128],
                rn32[:, 4 * ic:4 * ic + 4], id32[:],
            )
            nc.scalar.copy(
                rT16[:, ic * 128:(ic + 1) * 128],
                psO[ic][0:4, 0:128],
            )
        for t in range(12):
            junk_mm(psO[0])

        # ---- main loop: C tiles (PE) run 2 iterations ahead of the
        # mask-multiply (DVE) and aggregation matmuls so the PE never waits
        # on the DVE round-trip.
        LOOKAHEAD = 2
        cts = {}

        def emit_ct(jc):
            ct = ctp.tile([128, SH], F32, tag="ct", name=f"ct{jc}")
            nc.tensor.matmul(
                ct[:], wt[0:3, jc * 128:(jc + 1) * 128], rT16[0:3, :],
                start=True, stop=True,
            )
            cts[jc] = ct

        for jc in range(LOOKAHEAD):
            emit_ct(jc)
        for t in range(14):
            junk_mm(psO[1])
        for jc in range(JC):
            if jc + LOOKAHEAD < JC:
                emit_ct(jc + LOOKAHEAD)
            mt = mtp.tile([128, SH], F16, tag="mt", name=f"mt{jc}")
            nc.vector.scalar_tensor_tensor(
                mt[:], at8[:, jc * SH:(jc + 1) * SH], MTS, cts.pop(jc)[:],
                op0=mult, op1=mult,
            )
            for ic in range(IC):
                nc.tensor.matmul(
                    psO[ic][:],
                    mt[:, ic * 128:(ic + 1) * 128],
                    h16[:, jc * D:(jc + 1) * D],
                    start=(jc == 0),
                    stop=(jc == JC - 1),
                )

        # ---- store (fp16; host upcasts). Copies split ACT/DVE; the single
        # batched DMA lives on SP whose SEQ has no later work to block.
        ot = osb.tile([128, IC * D], F16, tag="ot")
        for ic in range(IC):
            dst = ot[:, ic * D:(ic + 1) * D]
            if ic % 2 == 0:
                nc.scalar.copy(dst, psO[ic][:])
            else:
                nc.vector.tensor_copy(dst, psO[ic][:])
        nc.sync.dma_start(
            out=out.rearrange("(ic p) d -> p ic d", p=128),
            in_=ot[:].rearrange("p (ic d) -> p ic d", d=D),
        )


_CACHE = {}


def _build1():
    if "p1" in _CACHE:
        return _CACHE["p1"]
    nc = bacc.Bacc("TRN2", target_bir_lowering=False, debug=False,
                   num_devices=NCORES)
    hp_in = nc.dram_tensor("hp_in", [D, HPW], F32, kind="ExternalInput").ap()
    w_out = nc.dram_tensor("w_out", [128, 12], F16, kind="ExternalOutput").ap()
    with tile.TileContext(nc) as tc:
        _body1(tc, hp_in, w_out)
    nc.compile()
    _CACHE["p1"] = nc
    return nc


def _build2():
    if "p2" in _CACHE:
        return _CACHE["p2"]
    nc = bacc.Bacc("TRN2", target_bir_lowering=False, debug=False,
                   num_devices=NCORES)
    at_in = nc.dram_tensor("at_in", [N, SH], F32, kind="ExternalInput").ap()
    h_in = nc.dram_tensor("h_in", [N, D], F32, kind="ExternalInput").ap()
    w4_in = nc.dram_tensor("w4_in", [128, JC * 4], F16,
                           kind="ExternalInput").ap()
    wt_in = nc.dram_tensor("wt_in", [4, N], F16, kind="ExternalInput").ap()
    id_in = nc.dram_tensor("id_in", [128, 128], F32, kind="ExternalInput").ap()
    out = nc.dram_tensor("out", [SH, D], F16, kind="ExternalOutput").ap()
    with tile.TileContext(nc) as tc:
        _body2(tc, at_in, h_in, w4_in, wt_in, id_in, out)
    nc.compile()
    _CACHE["p2"] = nc
    return nc


def kernel(graph_info, h, P, _trace=False, _results_out=None):
    graph_info = np.ascontiguousarray(graph_info, dtype=np.float32)
    h = np.ascontiguousarray(h, dtype=np.float32)
    P = np.ascontiguousarray(P, dtype=np.float32)
    nc1 = _build1()
    nc2 = _build2()

    # P1: hp = [h_shard.T | P | pad]
    pad = np.zeros((D, HPW - 512 - H), np.float32)
    in1 = [
        {"hp_in": np.ascontiguousarray(
            np.concatenate([h[c * SH:(c + 1) * SH, :].T, P, pad], axis=1))}
        for c in range(NCORES)
    ]
    res1 = bass_utils.run_bass_kernel_spmd(
        nc1, in1, core_ids=list(range(NCORES)), trace=_trace
    )
    w_full = np.concatenate(
        [res1.results[c]["w_out"].reshape(128, 4, 3).transpose(1, 0, 2)
         .reshape(SH, H) for c in range(NCORES)],
        axis=0,
    )

    # host packing of the tiny W' tables (pure data movement)
    w4_host = np.concatenate(
        [w_full.reshape(JC, 128, H).transpose(1, 0, 2),
         np.ones((128, JC, 1), np.float16)],
        axis=2,
    ).reshape(128, JC * 4)
    w4_host = np.ascontiguousarray(w4_host)
    wt_host = np.zeros((4, N), np.float16)
    wt_host[0:3, :] = w_full.T
    id_host = np.eye(128, dtype=np.float32)

    in2 = [
        {
            "at_in": np.ascontiguousarray(graph_info[c * SH:(c + 1) * SH, :].T),
            "h_in": h,
            "w4_in": w4_host,
            "wt_in": wt_host,
            "id_in": id_host,
        }
        for c in range(NCORES)
    ]
    res2 = bass_utils.run_bass_kernel_spmd(
        nc2, in2, core_ids=list(range(NCORES)), trace=_trace
    )
    if _results_out is not None:
        _results_out.extend([res1, res2])
    return np.concatenate(
        [res2.results[c]["out"].astype(np.float32) for c in range(NCORES)],
        axis=0,
    )
